# revision 12
# baseline (speedup 1.0000x reference)
"""Self-contained Trainium2 (Bass/Tile) kernel: single-step attention GRU decoder.

kernel(**inputs) takes the FULL inputs of setup_inputs() and returns
(log_softmax [1,V], hidden [1,1,H], attn_weights [1,L]) as fp32, matching the
reference. Internally it distributes across 8 NeuronCores:
  - vocab dim of the out projection sharded 8-ways (6400 rows/core, bf16)
  - GRU gate computation sharded by H-slice (128 rows of r,z,n per core)
  - tiny attention/combine stages replicated (fp32 attention for accuracy)
  - embedding lookup done host-side (single row gather; the table never
    touches the device)
  - log_softmax via per-core (max, sum) stats + AllGather
"""

import numpy as np
import ml_dtypes

import concourse.bass as bass
import concourse.bacc as bacc
import concourse.tile as tile
from concourse import mybir
from concourse.bass_utils import run_bass_kernel_spmd
from concourse.kernels.tile_matmul import make_identity

BF16 = ml_dtypes.bfloat16

P = 128
H = 1024
L = 512
V = 50257
NC = 8
VS = 6400            # padded vocab rows per core (8*6400 = 51200 >= V)
NVT = VS // P        # 50 vocab tiles per core
GS = 3 * P           # 384: this core's slice of (r,z,n) gates

F32 = mybir.dt.float32
BF = mybir.dt.bfloat16
AF = mybir.ActivationFunctionType
ALU = mybir.AluOpType
AX = mybir.AxisListType


def _build():
    nc = bacc.Bacc("TRN2", target_bir_lowering=False, debug=False, num_devices=NC)

    def inp(name, shape, dt):
        return nc.declare_dram_parameter(name, list(shape), dt, isOutput=False)

    def outp(name, shape, dt):
        return nc.declare_dram_parameter(name, list(shape), dt, isOutput=True)

    attn_in_col_d = inp("attn_in_col", [P, 16], BF)
    attn_wt_d = inp("attn_wt", [2 * H, L], BF)
    attn_b_d = inp("attn_b", [1, L], F32)
    enc_d = inp("enc", [L, H], BF)
    comb_wt_d = inp("comb_wt", [2 * H, H], BF)
    comb_b_d = inp("comb_b", [1, H], F32)
    emb_col_d = inp("emb_col", [P, 8], BF)
    gru_iht_d = inp("gru_iht", [H, GS], BF)
    gru_hht_d = inp("gru_hht", [H, GS], BF)
    gib_d = inp("gib", [1, GS], F32)
    ghb_d = inp("ghb", [1, GS], F32)
    h_col_d = inp("h_col", [P, 8], BF)
    h_slice_d = inp("h_slice", [1, P], F32)
    outw_d = inp("outw", [VS, H], BF)
    outb_col_d = inp("outb_col", [P, NVT], F32)

    logp_d = outp("logp_out", [NVT, P], F32)
    hnew_d = outp("hnew_out", [1, P], F32)
    attnw_d = outp("attnw_out", [1, L], F32)

    rg = [list(range(NC))]

    with tile.TileContext(nc) as tc:
        with (
            tc.tile_pool(name="const", bufs=1) as cp,
            tc.tile_pool(name="wp", bufs=1) as wp,
            tc.tile_pool(name="sp", bufs=1) as sp,
            tc.tile_pool(name="big", bufs=9) as bigp,
            tc.tile_pool(name="pp", bufs=2, space="PSUM") as pp,
            tc.tile_pool(name="dram", bufs=1, space="DRAM") as dp,
        ):
            ident = cp.tile([P, P], F32)
            make_identity(nc, ident)

            # ---- small-weight loads (emitted first so they win queue order)
            attn_in_sb = wp.tile([P, 16], BF)
            nc.gpsimd.dma_start(attn_in_sb[:], attn_in_col_d[:, :])
            attn_wt_sb = wp.tile([P, 16, L], BF)
            nc.gpsimd.dma_start(
                attn_wt_sb[:], attn_wt_d.ap().rearrange("(t p) n -> p t n", p=P)
            )
            attn_b_sb = wp.tile([1, L], F32)
            nc.gpsimd.dma_start(attn_b_sb[:], attn_b_d[:, :])
            enc_sb = wp.tile([P, 4, H], BF)
            nc.gpsimd.dma_start(enc_sb[:], enc_d.ap().rearrange("(t p) n -> p t n", p=P))
            comb_wt_sb = wp.tile([P, 16, H], BF)
            nc.gpsimd.dma_start(
                comb_wt_sb[:], comb_wt_d.ap().rearrange("(t p) n -> p t n", p=P)
            )
            comb_b_sb = wp.tile([1, H], F32)
            nc.gpsimd.dma_start(comb_b_sb[:], comb_b_d[:, :])
            emb_col_sb = wp.tile([P, 8], BF)
            nc.gpsimd.dma_start(emb_col_sb[:], emb_col_d[:, :])
            gru_iht_sb = wp.tile([P, 8, GS], BF)
            nc.gpsimd.dma_start(
                gru_iht_sb[:], gru_iht_d.ap().rearrange("(t p) n -> p t n", p=P)
            )
            gru_hht_sb = wp.tile([P, 8, GS], BF)
            nc.gpsimd.dma_start(
                gru_hht_sb[:], gru_hht_d.ap().rearrange("(t p) n -> p t n", p=P)
            )
            gib_sb = wp.tile([1, GS], F32)
            nc.gpsimd.dma_start(gib_sb[:], gib_d[:, :])
            ghb_sb = wp.tile([1, GS], F32)
            nc.gpsimd.dma_start(ghb_sb[:], ghb_d[:, :])
            h_col_sb = wp.tile([P, 8], BF)
            nc.gpsimd.dma_start(h_col_sb[:], h_col_d[:, :])
            h_slice_sb = wp.tile([1, P], F32)
            nc.gpsimd.dma_start(h_slice_sb[:], h_slice_d[:, :])
            outb_sb = wp.tile([P, NVT], F32)
            nc.gpsimd.dma_start(outb_sb[:], outb_col_d[:, :])

            # ---- big out_W stream: 12 groups of 4 [P,H] tiles + 1 group of 2
            groups = [4] * 12 + [2]
            outw_tiles = []
            row = 0
            for nt in groups:
                wt = bigp.tile([P, 4, H], BF, tag="outw", name="wt")
                nc.gpsimd.dma_start(
                    wt[:, 0:nt, :],
                    outw_d.ap()[row : row + nt * P, :].rearrange(
                        "(t p) n -> p t n", p=P
                    ),
                )
                outw_tiles.append((wt, nt))
                row += nt * P

            # ==== attention logits row [1, L] (fp32, replicated)
            ps_a = pp.tile([1, L], F32, tag="mm")
            for k in range(16):
                nc.tensor.matmul(
                    ps_a[:],
                    attn_in_sb[:, k : k + 1],
                    attn_wt_sb[:, k, :],
                    start=(k == 0),
                    stop=(k == 15),
                )
            la = sp.tile([1, L], F32)
            nc.vector.tensor_add(la[:], ps_a[:], attn_b_sb[:])

            # softmax over 512
            negm = sp.tile([1, 1], F32)
            nc.vector.tensor_reduce(negm[:], la[:], AX.X, ALU.max, negate=True)
            aw = sp.tile([1, L], F32)
            asum = sp.tile([1, 1], F32)
            nc.scalar.activation(aw[:], la[:], AF.Exp, bias=negm[:], accum_out=asum[:])
            ainv = sp.tile([1, 1], F32)
            nc.vector.reciprocal(ainv[:], asum[:])
            awn = sp.tile([1, L], F32)
            nc.vector.tensor_scalar_mul(awn[:], aw[:], ainv[:])
            nc.gpsimd.dma_start(attnw_d[:, :], awn[:])
            awb = sp.tile([1, L], BF)
            nc.vector.tensor_copy(awb[:], awn[:])

            # row -> column [P, 4] via DRAM round trip
            rt1 = dp.tile([1, L], BF)
            nc.gpsimd.dma_start(rt1[:, :], awb[:])
            aw_col = sp.tile([P, 4], BF)
            nc.gpsimd.dma_start(aw_col[:], rt1.rearrange("1 (t p) -> p t", p=P))

            # ==== attn_applied row [1, H] = attn_w @ enc
            app_b = sp.tile([1, H], BF)
            for half in range(2):
                ps_ap = pp.tile([1, L], F32, tag="mm", name="ps_ap")
                for k in range(4):
                    nc.tensor.matmul(
                        ps_ap[:],
                        aw_col[:, k : k + 1],
                        enc_sb[:, k, half * 512 : (half + 1) * 512],
                        start=(k == 0),
                        stop=(k == 3),
                    )
                nc.scalar.copy(app_b[0:1, half * 512 : (half + 1) * 512], ps_ap[:])
            rt2 = dp.tile([1, H], BF)
            nc.gpsimd.dma_start(rt2[:, :], app_b[:])
            app_col = sp.tile([P, 8], BF)
            nc.gpsimd.dma_start(app_col[:], rt2.rearrange("1 (t p) -> p t", p=P))

            # ==== combine + relu: x row [1, H] bf16
            xrow = sp.tile([1, H], BF)
            for half in range(2):
                ps_c = pp.tile([1, L], F32, tag="mm", name="ps_c")
                for k in range(16):
                    lhs = (
                        emb_col_sb[:, k : k + 1]
                        if k < 8
                        else app_col[:, k - 8 : k - 7]
                    )
                    nc.tensor.matmul(
                        ps_c[:],
                        lhs,
                        comb_wt_sb[:, k, half * 512 : (half + 1) * 512],
                        start=(k == 0),
                        stop=(k == 15),
                    )
                xs = sp.tile([1, L], F32, name="xs")
                nc.vector.tensor_add(
                    xs[:], ps_c[:], comb_b_sb[0:1, half * 512 : (half + 1) * 512]
                )
                nc.scalar.activation(
                    xrow[0:1, half * 512 : (half + 1) * 512], xs[:], AF.Relu
                )
            rt3 = dp.tile([1, H], BF)
            nc.gpsimd.dma_start(rt3[:, :], xrow[:])
            x_col = sp.tile([P, 8], BF)
            nc.gpsimd.dma_start(x_col[:], rt3.rearrange("1 (t p) -> p t", p=P))

            # ==== GRU slice: this core's 128 entries of each gate
            ps_gi = pp.tile([1, GS], F32, tag="gru")
            for k in range(8):
                nc.tensor.matmul(
                    ps_gi[:],
                    x_col[:, k : k + 1],
                    gru_iht_sb[:, k, :],
                    start=(k == 0),
                    stop=(k == 7),
                )
            ps_gh = pp.tile([1, GS], F32, tag="gru")
            for k in range(8):
                nc.tensor.matmul(
                    ps_gh[:],
                    h_col_sb[:, k : k + 1],
                    gru_hht_sb[:, k, :],
                    start=(k == 0),
                    stop=(k == 7),
                )
            gi = sp.tile([1, GS], F32)
            nc.vector.tensor_add(gi[:], ps_gi[:], gib_sb[:])
            gh = sp.tile([1, GS], F32)
            nc.vector.tensor_add(gh[:], ps_gh[:], ghb_sb[:])

            t_r = sp.tile([1, P], F32)
            nc.vector.tensor_add(t_r[:], gi[0:1, 0:P], gh[0:1, 0:P])
            r_g = sp.tile([1, P], F32)
            nc.scalar.activation(r_g[:], t_r[:], AF.Sigmoid)
            t_z = sp.tile([1, P], F32)
            nc.vector.tensor_add(t_z[:], gi[0:1, P : 2 * P], gh[0:1, P : 2 * P])
            z_g = sp.tile([1, P], F32)
            nc.scalar.activation(z_g[:], t_z[:], AF.Sigmoid)
            t_n = sp.tile([1, P], F32)
            nc.vector.tensor_mul(t_n[:], r_g[:], gh[0:1, 2 * P : 3 * P])
            t_n2 = sp.tile([1, P], F32)
            nc.vector.tensor_add(t_n2[:], gi[0:1, 2 * P : 3 * P], t_n[:])
            n_g = sp.tile([1, P], F32)
            nc.scalar.activation(n_g[:], t_n2[:], AF.Tanh)
            # h_new = n - z*n + z*h
            zn = sp.tile([1, P], F32)
            nc.vector.tensor_mul(zn[:], z_g[:], n_g[:])
            zh = sp.tile([1, P], F32)
            nc.vector.tensor_mul(zh[:], z_g[:], h_slice_sb[:])
            nm = sp.tile([1, P], F32)
            nc.vector.tensor_sub(nm[:], n_g[:], zn[:])
            hn = sp.tile([1, P], F32)
            nc.vector.tensor_add(hn[:], nm[:], zh[:])
            nc.gpsimd.dma_start(hnew_d[:, :], hn[:])

            # ==== AllGather h_new -> full [1, H] everywhere
            hn_in = dp.tile([1, P], F32)
            nc.gpsimd.dma_start(hn_in[:, :], hn[:])
            hn_all = dp.tile([1, H], F32)
            nc.gpsimd.collective_compute(
                "AllGather",
                ALU.bypass,
                replica_groups=rg,
                ins=[hn_in.opt()],
                outs=[hn_all.opt()],
            )
            hrow = sp.tile([1, H], F32)
            nc.gpsimd.dma_start(hrow[:], hn_all[:, :])
            hb = sp.tile([1, H], BF)
            nc.vector.tensor_copy(hb[:], hrow[:])
            hbc = sp.tile([P, H], BF)
            nc.gpsimd.partition_broadcast(hbc[:], hb[:])

            # ==== big matvec: logits column tile [P, NVT] (fused mult+reduce on DVE)
            lc = sp.tile([P, NVT], F32)
            junk = sp.tile([P, H], BF)
            j = 0
            for wt, nt in outw_tiles:
                for t in range(nt):
                    nc.vector.scalar_tensor_tensor(
                        out=junk[:],
                        in0=wt[:, t, :],
                        scalar=1.0,
                        in1=hbc[:],
                        op0=ALU.mult,
                        op1=ALU.mult,
                        accum_out=lc[:, j : j + 1],
                    )
                    j += 1
            lcb = sp.tile([P, NVT], F32)
            nc.vector.tensor_add(lcb[:], lc[:], outb_sb[:])

            # ==== local log-softmax stats (max, sum)
            mcol = sp.tile([P, 1], F32)
            nc.vector.tensor_reduce(mcol[:], lcb[:], AX.X, ALU.max)
            ps_t = pp.tile([1, P], F32, tag="tr")
            nc.tensor.transpose(ps_t[:], mcol[:], ident[:])
            m_l = sp.tile([1, 1], F32)
            nc.vector.tensor_reduce(m_l[:], ps_t[:], AX.X, ALU.max)
            negml = sp.tile([1, 1], F32)
            nc.vector.tensor_scalar_mul(negml[:], m_l[:], -1.0)
            negml_bc = sp.tile([P, 1], F32)
            nc.gpsimd.partition_broadcast(negml_bc[:], negml[:])
            ecol = sp.tile([P, NVT], F32)
            scol = sp.tile([P, 1], F32)
            nc.scalar.activation(
                ecol[:], lcb[:], AF.Exp, bias=negml_bc[:], accum_out=scol[:]
            )
            ps_t2 = pp.tile([1, P], F32, tag="tr")
            nc.tensor.transpose(ps_t2[:], scol[:], ident[:])
            s_l = sp.tile([1, 1], F32)
            nc.vector.tensor_reduce(s_l[:], ps_t2[:], AX.X, ALU.add)

            stats = sp.tile([1, 2], F32)
            nc.vector.tensor_copy(stats[0:1, 0:1], m_l[:])
            nc.vector.tensor_copy(stats[0:1, 1:2], s_l[:])
            st_in = dp.tile([1, 2], F32)
            nc.gpsimd.dma_start(st_in[:, :], stats[:])
            st_all = dp.tile([1, 2 * NC], F32)
            nc.gpsimd.collective_compute(
                "AllGather",
                ALU.bypass,
                replica_groups=rg,
                ins=[st_in.opt()],
                outs=[st_all.opt()],
            )
            sts = sp.tile([1, 2 * NC], F32)
            nc.gpsimd.dma_start(sts[:], st_all[:, :])
            msv = sp.tile([1, NC], F32)
            nc.vector.tensor_copy(
                msv[:], sts.rearrange("1 (r k) -> 1 k r", k=2)[0:1, 0, :]
            )
            ssv = sp.tile([1, NC], F32)
            nc.vector.tensor_copy(
                ssv[:], sts.rearrange("1 (r k) -> 1 k r", k=2)[0:1, 1, :]
            )

            negM = sp.tile([1, 1], F32)
            nc.vector.tensor_reduce(negM[:], msv[:], AX.X, ALU.max, negate=True)
            em = sp.tile([1, NC], F32)
            nc.scalar.activation(em[:], msv[:], AF.Exp, bias=negM[:])
            junk8 = sp.tile([1, NC], F32)
            S_g = sp.tile([1, 1], F32)
            nc.vector.scalar_tensor_tensor(
                out=junk8[:],
                in0=em[:],
                scalar=1.0,
                in1=ssv[:],
                op0=ALU.mult,
                op1=ALU.mult,
                accum_out=S_g[:],
            )
            lnS = sp.tile([1, 1], F32)
            nc.scalar.activation(lnS[:], S_g[:], AF.Ln)
            # logZ = M + ln(S) = lnS - negM
            logZ = sp.tile([1, 1], F32)
            nc.vector.tensor_sub(logZ[:], lnS[:], negM[:])
            logZ_bc = sp.tile([P, 1], F32)
            nc.gpsimd.partition_broadcast(logZ_bc[:], logZ[:])
            logp = sp.tile([P, NVT], F32)
            nc.vector.tensor_scalar_sub(logp[:], lcb[:], logZ_bc[:])

            ps_o = pp.tile([NVT, P], F32, tag="out", bufs=1)
            nc.tensor.transpose(ps_o[:], logp[:], ident[:])
            osb = sp.tile([NVT, P], F32)
            nc.scalar.copy(osb[:], ps_o[:])
            nc.gpsimd.dma_start(logp_d[:, :], osb[:])

    nc.compile()
    return nc


_nc_cache = None


def _get_nc():
    global _nc_cache
    if _nc_cache is None:
        _nc_cache = _build()
    return _nc_cache


def _prep_inmaps(
    input_tensor,
    hidden,
    encoder_outputs,
    emb,
    attn_W,
    attn_b,
    comb_W,
    comb_b,
    gru_w_ih,
    gru_w_hh,
    gru_b_ih,
    gru_b_hh,
    out_W,
    out_b,
):
    f32 = np.float32
    idx = int(np.asarray(input_tensor).reshape(-1)[0])
    emb = np.asarray(emb, f32)
    embedded = emb[idx]  # [H] -- the only row we need
    h = np.asarray(hidden, f32).reshape(H)
    attn_in = np.concatenate([embedded, h])  # [2H]
    attn_in_col = np.ascontiguousarray(attn_in.reshape(16, P).T).astype(BF16)
    attn_wt = np.ascontiguousarray(np.asarray(attn_W, f32).T).astype(BF16)  # [2H, L]
    attn_b_r = np.asarray(attn_b, f32).reshape(1, L)
    enc_bf = np.asarray(encoder_outputs, f32).astype(BF16)
    comb_wt = np.ascontiguousarray(np.asarray(comb_W, f32).T).astype(BF16)
    comb_b_r = np.asarray(comb_b, f32).reshape(1, H)
    emb_col = np.ascontiguousarray(embedded.reshape(8, P).T).astype(BF16)
    h_col = np.ascontiguousarray(h.reshape(8, P).T).astype(BF16)
    wih = np.asarray(gru_w_ih, f32)
    whh = np.asarray(gru_w_hh, f32)
    bih = np.asarray(gru_b_ih, f32)
    bhh = np.asarray(gru_b_hh, f32)
    out_W = np.asarray(out_W, f32)
    out_b = np.asarray(out_b, f32)

    in_maps = []
    for c in range(NC):
        sl = [slice(g * H + c * P, g * H + (c + 1) * P) for g in range(3)]
        iht = np.ascontiguousarray(
            np.concatenate([wih[s] for s in sl], axis=0).T
        ).astype(BF16)
        hht = np.ascontiguousarray(
            np.concatenate([whh[s] for s in sl], axis=0).T
        ).astype(BF16)
        gib = np.concatenate([bih[s] for s in sl]).reshape(1, GS).astype(f32)
        ghb = np.concatenate([bhh[s] for s in sl]).reshape(1, GS).astype(f32)
        h_slice = np.ascontiguousarray(h[c * P : (c + 1) * P]).reshape(1, P)
        r0, r1 = c * VS, min((c + 1) * VS, V)
        wshard = np.zeros((VS, H), dtype=BF16)
        wshard[: r1 - r0] = out_W[r0:r1].astype(BF16)
        bshard = np.full(VS, -1e30, dtype=f32)
        bshard[: r1 - r0] = out_b[r0:r1]
        outb_col = np.ascontiguousarray(bshard.reshape(NVT, P).T)
        in_maps.append(
            {
                "attn_in_col": attn_in_col,
                "attn_wt": attn_wt,
                "attn_b": attn_b_r,
                "enc": enc_bf,
                "comb_wt": comb_wt,
                "comb_b": comb_b_r,
                "emb_col": emb_col,
                "gru_iht": iht,
                "gru_hht": hht,
                "gib": gib,
                "ghb": ghb,
                "h_col": h_col,
                "h_slice": h_slice,
                "outw": wshard,
                "outb_col": outb_col,
            }
        )
    return in_maps


def _assemble(res):
    logp = np.concatenate([np.asarray(r["logp_out"]).reshape(-1) for r in res])[:V]
    logp = logp.reshape(1, V).astype(np.float32)
    hn = (
        np.concatenate([np.asarray(r["hnew_out"]).reshape(-1) for r in res])
        .reshape(1, 1, H)
        .astype(np.float32)
    )
    aw = np.asarray(res[0]["attnw_out"]).reshape(1, L).astype(np.float32)
    return logp, hn, aw


def kernel(**inputs):
    nc = _get_nc()
    in_maps = _prep_inmaps(**inputs)
    res = run_bass_kernel_spmd(nc, in_maps, list(range(NC))).results
    return _assemble(res)
